# revision 52
# baseline (speedup 1.0000x reference)
"""CamCenterLoss (segment-mean SmoothL1) on 8 Trainium2 NeuronCores.

Sharding: each (label, cam) segment is assigned wholly to one core.
Segments (size>=2; singletons contribute 0) are packed into 128-row
blocks with best-fit-decreasing, and blocks are dealt across the 8
cores so every core gets the same block count (nblk ~ 14).

Per block the device computes d = M^T @ fe where M = P - I is the
block-local averaging projector built on the host (P[i,j] = 1/c if
rows i,j in the same segment else 0), so targets - feats needs ONE
[128x128] @ [128x2048] matmul per block.

SmoothL1 identity used on device (a = |d|, m = min(a, 1)):
    sl1 = a - (m - 0.5*m^2) = a + 0.5*((m - 2) * m)
so the loss partial needs only two sums, each riding a fused op:
  PE  : d = M^T @ fe                      (4 matmuls of N=512, 1 PSUM tile)
  ACT : a = Abs(d), accum Sum_a           (drains PSUM -> bf16 SBUF)
  DVE : m = min(a, 1)                     (tensor_scalar, 4x mode)
  DVE : v = (m sub 2) mult m, accum Sum_v (scalar_tensor_tensor)
  partial = Sum_a + 0.5 * Sum_v
(tensor_tensor_reduce dies at runtime on HW; tensor_scalar with accum_out
drops to 1x mode -- hence exactly one accum-free 4x op plus one stt.)

A slice of each block's v columns is handed to the Scalar engine as
w = (m - 1)^2 = v + 1 (Square with bias; Sum_v = Sum_w - count): 128
columns at steady state, and 1024-1280 columns on the last three blocks
where the DVE is otherwise the lone drain bottleneck.
"""

import numpy as np
import ml_dtypes

N_CORES = 8
NUM_CAMS = 8
NUM_LABELS = 1024
D_FEAT = 2048
QCHUNK = 512
WCOL_STEADY = 128      # per-block v columns handed to ACT (w = (m-1)^2)


# ----------------------------------------------------------------------------
# Host-side preprocessing (index manipulation + row permutation + dtype cast)
# ----------------------------------------------------------------------------

def _preprocess(feats, labels, cam_ids):
    feats = np.ascontiguousarray(np.asarray(feats, dtype=np.float32))
    labels = np.asarray(labels).astype(np.int64)
    cams = np.asarray(cam_ids).astype(np.int64)
    N, D = feats.shape

    # Global segment id; gather row lists per segment with one argsort.
    seg = labels * NUM_CAMS + cams
    order = np.argsort(seg, kind="stable")
    seg_sorted = seg[order]
    starts = np.flatnonzero(np.r_[True, seg_sorted[1:] != seg_sorted[:-1]])
    ends = np.r_[starts[1:], N]
    # Keep segments with >= 2 rows; singletons have d == 0.
    runs = [(e - s, s) for s, e in zip(starts, ends) if e - s >= 2]
    if any(rl > 128 for rl, _ in runs):
        raise ValueError("segment with more than 128 rows")

    # Best-fit-decreasing pack into 128-row bins.
    runs.sort(reverse=True)
    bins = []          # list of (used, [(start, len), ...])
    for rl, s in runs:
        best_i, best_used = -1, -1
        for i, (used, _) in enumerate(bins):
            if used + rl <= 128 and used > best_used:
                best_i, best_used = i, used
        if best_i < 0:
            bins.append((rl, [(s, rl)]))
        else:
            used, lst = bins[best_i]
            lst.append((s, rl))
            bins[best_i] = (used + rl, lst)

    nbins = len(bins)
    nblk = -(-nbins // N_CORES)
    nblk = max(nblk, 1)

    bf16 = ml_dtypes.bfloat16
    fp8 = ml_dtypes.float8_e4m3fn
    feats_s = np.zeros((N_CORES, nblk * 128, D), dtype=fp8)
    m_mat32 = np.zeros((N_CORES, nblk, 128, 128), dtype=np.float32)

    for i, (_, lst) in enumerate(bins):
        c, b = i % N_CORES, i // N_CORES
        k = 0
        for (s, rl) in lst:
            ridx = order[s:s + rl]
            feats_s[c, 128 * b + k:128 * b + k + rl] = feats[ridx]
            blk = m_mat32[c, b]
            blk[k:k + rl, k:k + rl] = 1.0 / rl
            for j in range(k, k + rl):
                blk[j, j] -= 1.0
            k += rl
    m_mat = m_mat32.astype(bf16)
    return feats_s, m_mat, nblk, N, D


# ----------------------------------------------------------------------------
# Device program
# ----------------------------------------------------------------------------

def _build_program(nblk, D):
    import concourse.bacc as bacc
    import concourse.mybir as mybir
    import concourse.tile as tile

    dt = mybir.dt
    f32, bf16, f8 = dt.float32, dt.bfloat16, dt.float8e4
    Alu = mybir.AluOpType
    Act = mybir.ActivationFunctionType

    nc = bacc.Bacc("TRN2", target_bir_lowering=False, debug=False,
                   num_devices=N_CORES)
    feats_d = nc.dram_tensor("feats_s", [nblk * 128, D], f8,
                             kind="ExternalInput").ap()
    mmat_d = nc.dram_tensor("m_mat", [nblk, 128, 128], bf16,
                            kind="ExternalInput").ap()
    out_d = nc.dram_tensor("partial", [1, 1], f32, kind="ExternalOutput").ap()

    with tile.TileContext(nc) as tc:
        with (
            tc.tile_pool(name="const", bufs=1) as const_pool,
            tc.tile_pool(name="feats", bufs=3) as feats_pool,
            tc.tile_pool(name="wts", bufs=3) as wts_pool,
            tc.tile_pool(name="aa", bufs=2) as a_pool,
            tc.tile_pool(name="tt", bufs=2) as t_pool,
            tc.tile_pool(name="uu", bufs=2) as u_pool,
            tc.tile_pool(name="psumd", bufs=2, space="PSUM") as psum_d_pool,
        ):
            stats_a = const_pool.tile([128, nblk], f32, tag="stats_a")
            stats_v = const_pool.tile([128, nblk], f32, tag="stats_v")
            stats_w = const_pool.tile([128, nblk], f32, tag="stats_w")
            nc.vector.memset(stats_w[:], 0.0)
            ones1 = const_pool.tile([128, 1], f32, tag="ones1")
            nc.gpsimd.memset(ones1[:], 1.0)
            negone = const_pool.tile([128, 1], f32, tag="negone")
            nc.gpsimd.memset(negone[:], -1.0)

            for b in range(nblk):
                fe = feats_pool.tile([128, D], f8, tag="fe")
                nc.sync.dma_start(fe[:], feats_d[128 * b:128 * (b + 1), :])
                mt = wts_pool.tile([128, 128], bf16, tag="mt")
                nc.sync.dma_start(mt[:], mmat_d[b])

                dps = psum_d_pool.tile([128, D], f32, tag="d")
                for q in range(D // QCHUNK):
                    sl = slice(q * QCHUNK, (q + 1) * QCHUNK)
                    nc.tensor.matmul(dps[:, sl], mt[:], fe[:, sl],
                                     start=True, stop=True)

                a = a_pool.tile([128, D], bf16, tag="a")
                nc.scalar.activation(a[:], dps[:], Act.Abs,
                                     accum_out=stats_a[:, b:b + 1])

                # m = min(a, 1)  (4x mode: bf16, SBUF, no accum)
                m = t_pool.tile([128, D], bf16, tag="m")
                nc.vector.tensor_scalar_min(m[:], a[:], 1.0)

                # v = (m - 2) * m; accum -> Sum_v   (v itself is dead)
                # Split v columns with ACT (which has slack) via
                # w = (m - 1)^2 = v + 1  =>  Sum_v = Sum_w - count.
                # Tail blocks split half/half: DVE is the drain bottleneck at
                # the end of the kernel while ACT idles.
                if b >= nblk - 2:
                    wcols = 1280
                elif b == nblk - 3:
                    wcols = D // 2
                elif b == nblk - 4:
                    wcols = 512
                else:
                    wcols = WCOL_STEADY
                split = D - wcols
                v = u_pool.tile([128, split], bf16, tag="v")
                nc.vector.scalar_tensor_tensor(
                    v[:], m[:, 0:split], 2.0, m[:, 0:split],
                    op0=Alu.subtract, op1=Alu.mult,
                    accum_out=stats_v[:, b:b + 1])
                if wcols:
                    w = u_pool.tile([128, wcols], bf16, tag="w")
                    nc.scalar.activation(
                        w[:], m[:, split:D], Act.Square, bias=negone[:],
                        accum_out=stats_w[:, b:b + 1])

            # partial = Sum_a + 0.5 * (Sum_v + Sum_w - wtot), per partition
            red_a = const_pool.tile([128, 1], f32, tag="red_a")
            nc.vector.tensor_reduce(red_a[:], stats_a[:],
                                    axis=mybir.AxisListType.X, op=Alu.add)
            red_v = const_pool.tile([128, 1], f32, tag="red_v")
            nc.vector.tensor_reduce(red_v[:], stats_v[:],
                                    axis=mybir.AxisListType.X, op=Alu.add)
            red_w = const_pool.tile([128, 1], f32, tag="red_w")
            nc.vector.tensor_reduce(red_w[:], stats_w[:],
                                    axis=mybir.AxisListType.X, op=Alu.add)
            wtot = WCOL_STEADY * (nblk - 4) + 2 * 1280 + D // 2 + 512
            red_vw = const_pool.tile([128, 1], f32, tag="red_vw")
            nc.vector.scalar_tensor_tensor(red_vw[:], red_w[:], -float(wtot),
                                           red_v[:], op0=Alu.add, op1=Alu.add)
            red = const_pool.tile([128, 1], f32, tag="red")
            nc.vector.scalar_tensor_tensor(red[:], red_vw[:], 0.5,
                                           red_a[:], op0=Alu.mult,
                                           op1=Alu.add)
            fin = psum_d_pool.tile([1, 1], f32, tag="d")
            nc.tensor.matmul(fin[:], red[:], ones1[:], start=True, stop=True)
            outsb = const_pool.tile([1, 1], f32, tag="outsb")
            nc.scalar.copy(outsb[:], fin[:])
            nc.sync.dma_start(out_d[:], outsb[:])

    nc.compile()
    return nc


_PROGRAM_CACHE = {}


def _get_program(nblk, D):
    key = (nblk, D)
    if key not in _PROGRAM_CACHE:
        _PROGRAM_CACHE[key] = _build_program(nblk, D)
    return _PROGRAM_CACHE[key]


def make_in_maps(feats, labels, cam_ids):
    """Host shard + program build; returns (nc, in_maps, N, D)."""
    feats_s, m_mat, nblk, N, D = _preprocess(feats, labels, cam_ids)
    nc = _get_program(nblk, D)
    in_maps = [
        {"feats_s": feats_s[c], "m_mat": m_mat[c]}
        for c in range(N_CORES)
    ]
    return nc, in_maps, N, D


def kernel(feats, labels, cam_ids):
    from concourse.bass_utils import run_bass_kernel_spmd

    nc, in_maps, N, D = make_in_maps(feats, labels, cam_ids)
    res = run_bass_kernel_spmd(nc, in_maps, core_ids=list(range(N_CORES)))
    total = np.sum(
        np.array([res.results[c]["partial"][0, 0] for c in range(N_CORES)],
                 dtype=np.float64))
    return np.float32(total / (float(N) * float(D)))


# revision 58
# speedup vs baseline: 1.0167x; 1.0167x over previous
"""CamCenterLoss (segment-mean SmoothL1) on 8 Trainium2 NeuronCores.

Sharding: each (label, cam) segment is assigned wholly to one core.
Segments (size>=2; singletons contribute 0) are packed into 128-row
blocks with best-fit-decreasing, and blocks are dealt across the 8
cores so every core gets the same block count (nblk ~ 14).

Per block the device computes d = M^T @ fe where M = P - I is the
block-local averaging projector built on the host (P[i,j] = 1/c if
rows i,j in the same segment else 0), so targets - feats needs ONE
[128x128] @ [128x2048] matmul per block.

SmoothL1 identity used on device (a = |d|, m = min(a, 1)):
    sl1 = a - (m - 0.5*m^2) = a + 0.5*((m - 2) * m)
so the loss partial needs only two sums, each riding a fused op:
  PE  : d = M^T @ fe                      (4 matmuls of N=512, 1 PSUM tile)
  ACT : a = Abs(d), accum Sum_a           (drains PSUM -> bf16 SBUF)
  DVE : m = min(a, 1)                     (tensor_scalar, 4x mode)
  DVE : v = (m sub 2) mult m, accum Sum_v (scalar_tensor_tensor)
  partial = Sum_a + 0.5 * Sum_v
(tensor_tensor_reduce dies at runtime on HW; tensor_scalar with accum_out
drops to 1x mode -- hence exactly one accum-free 4x op plus one stt.)

A slice of each block's v columns is handed to the Scalar engine as
w = (m - 1)^2 = v + 1 (Square with bias; Sum_v = Sum_w - count): 128
columns at steady state, and 1024-1280 columns on the last three blocks
where the DVE is otherwise the lone drain bottleneck.
"""

import numpy as np
import ml_dtypes

N_CORES = 8
NUM_CAMS = 8
NUM_LABELS = 1024
D_FEAT = 2048
QCHUNK = 512
WCOL_STEADY = 128      # per-block v columns handed to ACT (w = (m-1)^2)


# ----------------------------------------------------------------------------
# Host-side preprocessing (index manipulation + row permutation + dtype cast)
# ----------------------------------------------------------------------------

def _preprocess(feats, labels, cam_ids):
    feats = np.ascontiguousarray(np.asarray(feats, dtype=np.float32))
    labels = np.asarray(labels).astype(np.int64)
    cams = np.asarray(cam_ids).astype(np.int64)
    N, D = feats.shape

    # Global segment id; gather row lists per segment with one argsort.
    seg = labels * NUM_CAMS + cams
    order = np.argsort(seg, kind="stable")
    seg_sorted = seg[order]
    starts = np.flatnonzero(np.r_[True, seg_sorted[1:] != seg_sorted[:-1]])
    ends = np.r_[starts[1:], N]
    # Keep segments with >= 2 rows; singletons have d == 0.
    runs = [(e - s, s) for s, e in zip(starts, ends) if e - s >= 2]
    if any(rl > 128 for rl, _ in runs):
        raise ValueError("segment with more than 128 rows")

    # Best-fit-decreasing pack into 128-row bins.
    runs.sort(reverse=True)
    bins = []          # list of (used, [(start, len), ...])
    for rl, s in runs:
        best_i, best_used = -1, -1
        for i, (used, _) in enumerate(bins):
            if used + rl <= 128 and used > best_used:
                best_i, best_used = i, used
        if best_i < 0:
            bins.append((rl, [(s, rl)]))
        else:
            used, lst = bins[best_i]
            lst.append((s, rl))
            bins[best_i] = (used + rl, lst)

    nbins = len(bins)
    nblk = -(-nbins // N_CORES)
    nblk = max(nblk, 1)

    bf16 = ml_dtypes.bfloat16
    fp8 = ml_dtypes.float8_e4m3fn
    feats_s = np.zeros((N_CORES, nblk * 128, D), dtype=fp8)
    m_mat32 = np.zeros((N_CORES, nblk, 128, 128), dtype=np.float32)

    for i, (_, lst) in enumerate(bins):
        c, b = i % N_CORES, i // N_CORES
        k = 0
        for (s, rl) in lst:
            ridx = order[s:s + rl]
            feats_s[c, 128 * b + k:128 * b + k + rl] = feats[ridx]
            blk = m_mat32[c, b]
            blk[k:k + rl, k:k + rl] = 1.0 / rl
            for j in range(k, k + rl):
                blk[j, j] -= 1.0
            k += rl
    m_mat = m_mat32.astype(bf16)

    # Embed each block's M matrix (bf16, 256 raw bytes/row) after the D
    # fp8 feature columns, so one DMA per block carries both; the device
    # bitcasts the tail slice back to bf16 for LDWEIGHTS.
    feats_ext = np.zeros((N_CORES, nblk * 128, D + 256), dtype=fp8)
    feats_ext[:, :, :D] = feats_s
    me = feats_ext.view(np.uint8)
    for c in range(N_CORES):
        for b in range(nblk):
            me[c, 128 * b:128 * (b + 1), D:D + 256] = \
                m_mat[c, b].view(np.uint8)
    return feats_ext, m_mat, nblk, N, D


# ----------------------------------------------------------------------------
# Device program
# ----------------------------------------------------------------------------

def _build_program(nblk, D):
    import concourse.bacc as bacc
    import concourse.mybir as mybir
    import concourse.tile as tile

    dt = mybir.dt
    f32, bf16, f8 = dt.float32, dt.bfloat16, dt.float8e4
    Alu = mybir.AluOpType
    Act = mybir.ActivationFunctionType

    nc = bacc.Bacc("TRN2", target_bir_lowering=False, debug=False,
                   num_devices=N_CORES)
    feats_d = nc.dram_tensor("feats_s", [nblk * 128, D + 256], f8,
                             kind="ExternalInput").ap()
    out_d = nc.dram_tensor("partial", [1, 1], f32, kind="ExternalOutput").ap()

    with tile.TileContext(nc) as tc:
        with (
            tc.tile_pool(name="const", bufs=1) as const_pool,
            tc.tile_pool(name="feats", bufs=3) as feats_pool,
            tc.tile_pool(name="wts", bufs=3) as wts_pool,
            tc.tile_pool(name="aa", bufs=2) as a_pool,
            tc.tile_pool(name="tt", bufs=2) as t_pool,
            tc.tile_pool(name="uu", bufs=2) as u_pool,
            tc.tile_pool(name="psumd", bufs=2, space="PSUM") as psum_d_pool,
        ):
            stats_a = const_pool.tile([128, nblk], f32, tag="stats_a")
            stats_v = const_pool.tile([128, nblk], f32, tag="stats_v")
            stats_w = const_pool.tile([128, nblk], f32, tag="stats_w")
            nc.vector.memset(stats_w[:], 0.0)
            ones1 = const_pool.tile([128, 1], f32, tag="ones1")
            nc.gpsimd.memset(ones1[:], 1.0)
            negone = const_pool.tile([128, 1], f32, tag="negone")
            nc.gpsimd.memset(negone[:], -1.0)

            for b in range(nblk):
                fe = feats_pool.tile([128, D + 256], f8, tag="fe")
                nc.sync.dma_start(fe[:], feats_d[128 * b:128 * (b + 1), :])
                mt = fe[:, D:D + 256].bitcast(bf16)

                dps = psum_d_pool.tile([128, D], f32, tag="d")
                for q in range(D // QCHUNK):
                    sl = slice(q * QCHUNK, (q + 1) * QCHUNK)
                    nc.tensor.matmul(dps[:, sl], mt, fe[:, sl],
                                     start=True, stop=True)

                a = a_pool.tile([128, D], bf16, tag="a")
                nc.scalar.activation(a[:], dps[:], Act.Abs,
                                     accum_out=stats_a[:, b:b + 1])

                # m = min(a, 1)  (4x mode: bf16, SBUF, no accum)
                m = t_pool.tile([128, D], bf16, tag="m")
                nc.vector.tensor_scalar_min(m[:], a[:], 1.0)

                # v = (m - 2) * m; accum -> Sum_v   (v itself is dead)
                # Split v columns with ACT (which has slack) via
                # w = (m - 1)^2 = v + 1  =>  Sum_v = Sum_w - count.
                # Tail blocks split half/half: DVE is the drain bottleneck at
                # the end of the kernel while ACT idles.
                if b >= nblk - 2:
                    wcols = 1280
                elif b == nblk - 3:
                    wcols = D // 2
                else:
                    wcols = WCOL_STEADY
                split = D - wcols
                v = u_pool.tile([128, split], bf16, tag="v")
                nc.vector.scalar_tensor_tensor(
                    v[:], m[:, 0:split], 2.0, m[:, 0:split],
                    op0=Alu.subtract, op1=Alu.mult,
                    accum_out=stats_v[:, b:b + 1])
                if wcols:
                    w = u_pool.tile([128, wcols], bf16, tag="w")
                    nc.scalar.activation(
                        w[:], m[:, split:D], Act.Square, bias=negone[:],
                        accum_out=stats_w[:, b:b + 1])

            # partial = Sum_a + 0.5 * (Sum_v + Sum_w - wtot), per partition
            red_a = const_pool.tile([128, 1], f32, tag="red_a")
            nc.vector.tensor_reduce(red_a[:], stats_a[:],
                                    axis=mybir.AxisListType.X, op=Alu.add)
            red_v = const_pool.tile([128, 1], f32, tag="red_v")
            nc.vector.tensor_reduce(red_v[:], stats_v[:],
                                    axis=mybir.AxisListType.X, op=Alu.add)
            red_w = const_pool.tile([128, 1], f32, tag="red_w")
            nc.vector.tensor_reduce(red_w[:], stats_w[:],
                                    axis=mybir.AxisListType.X, op=Alu.add)
            wtot = WCOL_STEADY * (nblk - 3) + 2 * 1280 + D // 2
            red_vw = const_pool.tile([128, 1], f32, tag="red_vw")
            nc.vector.scalar_tensor_tensor(red_vw[:], red_w[:], -float(wtot),
                                           red_v[:], op0=Alu.add, op1=Alu.add)
            red = const_pool.tile([128, 1], f32, tag="red")
            nc.vector.scalar_tensor_tensor(red[:], red_vw[:], 0.5,
                                           red_a[:], op0=Alu.mult,
                                           op1=Alu.add)
            fin = psum_d_pool.tile([1, 1], f32, tag="d")
            nc.tensor.matmul(fin[:], red[:], ones1[:], start=True, stop=True)
            outsb = const_pool.tile([1, 1], f32, tag="outsb")
            nc.scalar.copy(outsb[:], fin[:])
            nc.sync.dma_start(out_d[:], outsb[:])

    nc.compile()
    return nc


_PROGRAM_CACHE = {}


def _get_program(nblk, D):
    key = (nblk, D)
    if key not in _PROGRAM_CACHE:
        _PROGRAM_CACHE[key] = _build_program(nblk, D)
    return _PROGRAM_CACHE[key]


def make_in_maps(feats, labels, cam_ids):
    """Host shard + program build; returns (nc, in_maps, N, D)."""
    feats_s, m_mat, nblk, N, D = _preprocess(feats, labels, cam_ids)
    nc = _get_program(nblk, D)
    in_maps = [
        {"feats_s": feats_s[c]}
        for c in range(N_CORES)
    ]
    return nc, in_maps, N, D


def kernel(feats, labels, cam_ids):
    from concourse.bass_utils import run_bass_kernel_spmd

    nc, in_maps, N, D = make_in_maps(feats, labels, cam_ids)
    res = run_bass_kernel_spmd(nc, in_maps, core_ids=list(range(N_CORES)))
    total = np.sum(
        np.array([res.results[c]["partial"][0, 0] for c in range(N_CORES)],
                 dtype=np.float64))
    return np.float32(total / (float(N) * float(D)))


# revision 59
# speedup vs baseline: 1.0188x; 1.0021x over previous
"""CamCenterLoss (segment-mean SmoothL1) on 8 Trainium2 NeuronCores.

Sharding: each (label, cam) segment is assigned wholly to one core.
Segments (size>=2; singletons contribute 0) are packed into 128-row
blocks with best-fit-decreasing, and blocks are dealt across the 8
cores so every core gets the same block count (nblk ~ 14).

Per block the device computes d = M^T @ fe where M = P - I is the
block-local averaging projector built on the host (P[i,j] = 1/c if
rows i,j in the same segment else 0), so targets - feats needs ONE
[128x128] @ [128x2048] matmul per block.

SmoothL1 identity used on device (a = |d|, m = min(a, 1)):
    sl1 = a - (m - 0.5*m^2) = a + 0.5*((m - 2) * m)
so the loss partial needs only two sums, each riding a fused op:
  PE  : d = M^T @ fe                      (4 matmuls of N=512, 1 PSUM tile)
  ACT : a = Abs(d), accum Sum_a           (drains PSUM -> bf16 SBUF)
  DVE : m = min(a, 1)                     (tensor_scalar, 4x mode)
  DVE : v = (m sub 2) mult m, accum Sum_v (scalar_tensor_tensor)
  partial = Sum_a + 0.5 * Sum_v
(tensor_tensor_reduce dies at runtime on HW; tensor_scalar with accum_out
drops to 1x mode -- hence exactly one accum-free 4x op plus one stt.)

A slice of each block's v columns is handed to the Scalar engine as
w = (m - 1)^2 = v + 1 (Square with bias; Sum_v = Sum_w - count): 128
columns at steady state, and 1024-1280 columns on the last three blocks
where the DVE is otherwise the lone drain bottleneck.
"""

import numpy as np
import ml_dtypes

N_CORES = 8
NUM_CAMS = 8
NUM_LABELS = 1024
D_FEAT = 2048
QCHUNK = 512
WCOL_STEADY = 128      # per-block v columns handed to ACT (w = (m-1)^2)


# ----------------------------------------------------------------------------
# Host-side preprocessing (index manipulation + row permutation + dtype cast)
# ----------------------------------------------------------------------------

def _preprocess(feats, labels, cam_ids):
    feats = np.ascontiguousarray(np.asarray(feats, dtype=np.float32))
    labels = np.asarray(labels).astype(np.int64)
    cams = np.asarray(cam_ids).astype(np.int64)
    N, D = feats.shape

    # Global segment id; gather row lists per segment with one argsort.
    seg = labels * NUM_CAMS + cams
    order = np.argsort(seg, kind="stable")
    seg_sorted = seg[order]
    starts = np.flatnonzero(np.r_[True, seg_sorted[1:] != seg_sorted[:-1]])
    ends = np.r_[starts[1:], N]
    # Keep segments with >= 2 rows; singletons have d == 0.
    runs = [(e - s, s) for s, e in zip(starts, ends) if e - s >= 2]
    if any(rl > 128 for rl, _ in runs):
        raise ValueError("segment with more than 128 rows")

    # Best-fit-decreasing pack into 128-row bins.
    runs.sort(reverse=True)
    bins = []          # list of (used, [(start, len), ...])
    for rl, s in runs:
        best_i, best_used = -1, -1
        for i, (used, _) in enumerate(bins):
            if used + rl <= 128 and used > best_used:
                best_i, best_used = i, used
        if best_i < 0:
            bins.append((rl, [(s, rl)]))
        else:
            used, lst = bins[best_i]
            lst.append((s, rl))
            bins[best_i] = (used + rl, lst)

    nbins = len(bins)
    nblk = -(-nbins // N_CORES)
    nblk = max(nblk, 1)

    bf16 = ml_dtypes.bfloat16
    fp8 = ml_dtypes.float8_e4m3fn
    feats_s = np.zeros((N_CORES, nblk * 128, D), dtype=fp8)
    m_mat32 = np.zeros((N_CORES, nblk, 128, 128), dtype=np.float32)

    for i, (_, lst) in enumerate(bins):
        c, b = i % N_CORES, i // N_CORES
        k = 0
        for (s, rl) in lst:
            ridx = order[s:s + rl]
            feats_s[c, 128 * b + k:128 * b + k + rl] = feats[ridx]
            blk = m_mat32[c, b]
            blk[k:k + rl, k:k + rl] = 1.0 / rl
            for j in range(k, k + rl):
                blk[j, j] -= 1.0
            k += rl
    m_mat = m_mat32.astype(bf16)

    # Embed each block's M matrix (bf16, 256 raw bytes/row) after the D
    # fp8 feature columns, so one DMA per block carries both; the device
    # bitcasts the tail slice back to bf16 for LDWEIGHTS.
    feats_ext = np.zeros((N_CORES, nblk * 128, D + 256), dtype=fp8)
    feats_ext[:, :, :D] = feats_s
    me = feats_ext.view(np.uint8)
    for c in range(N_CORES):
        for b in range(nblk):
            me[c, 128 * b:128 * (b + 1), D:D + 256] = \
                m_mat[c, b].view(np.uint8)
    return feats_ext, m_mat, nblk, N, D


# ----------------------------------------------------------------------------
# Device program
# ----------------------------------------------------------------------------

def _build_program(nblk, D):
    import concourse.bacc as bacc
    import concourse.mybir as mybir
    import concourse.tile as tile

    dt = mybir.dt
    f32, bf16, f8 = dt.float32, dt.bfloat16, dt.float8e4
    Alu = mybir.AluOpType
    Act = mybir.ActivationFunctionType

    nc = bacc.Bacc("TRN2", target_bir_lowering=False, debug=False,
                   num_devices=N_CORES)
    feats_d = nc.dram_tensor("feats_s", [nblk * 128, D + 256], f8,
                             kind="ExternalInput").ap()
    out_d = nc.dram_tensor("partial", [1, 1], f32, kind="ExternalOutput").ap()

    with tile.TileContext(nc) as tc:
        with (
            tc.tile_pool(name="const", bufs=1) as const_pool,
            tc.tile_pool(name="feats", bufs=3) as feats_pool,
            tc.tile_pool(name="wts", bufs=3) as wts_pool,
            tc.tile_pool(name="aa", bufs=2) as a_pool,
            tc.tile_pool(name="tt", bufs=2) as t_pool,
            tc.tile_pool(name="uu", bufs=2) as u_pool,
            tc.tile_pool(name="psumd", bufs=2, space="PSUM") as psum_d_pool,
        ):
            stats_a = const_pool.tile([128, nblk], f32, tag="stats_a")
            stats_v = const_pool.tile([128, nblk], f32, tag="stats_v")
            stats_w = const_pool.tile([128, nblk], f32, tag="stats_w")
            nc.vector.memset(stats_w[:], 0.0)
            ones1 = const_pool.tile([128, 1], f32, tag="ones1")
            nc.gpsimd.memset(ones1[:], 1.0)
            negone = const_pool.tile([128, 1], f32, tag="negone")
            nc.gpsimd.memset(negone[:], -1.0)
            # Dummy ACTIVATE at t~0 pulls the ~2.7us ACT_TABLE_LOAD (walrus
            # inserts it before the first ACTIVATE) under the initial DMA
            # wait instead of serializing it before the first real ABS.
            dummy = const_pool.tile([128, 1], bf16, tag="dummy")
            nc.scalar.activation(dummy[:], ones1[:], Act.Abs)

            for b in range(nblk):
                fe = feats_pool.tile([128, D + 256], f8, tag="fe")
                nc.sync.dma_start(fe[:], feats_d[128 * b:128 * (b + 1), :])
                mt = fe[:, D:D + 256].bitcast(bf16)

                dps = psum_d_pool.tile([128, D], f32, tag="d")
                for q in range(D // QCHUNK):
                    sl = slice(q * QCHUNK, (q + 1) * QCHUNK)
                    nc.tensor.matmul(dps[:, sl], mt, fe[:, sl],
                                     start=True, stop=True)

                a = a_pool.tile([128, D], bf16, tag="a")
                nc.scalar.activation(a[:], dps[:], Act.Abs,
                                     accum_out=stats_a[:, b:b + 1])

                # m = min(a, 1)  (4x mode: bf16, SBUF, no accum)
                m = t_pool.tile([128, D], bf16, tag="m")
                nc.vector.tensor_scalar_min(m[:], a[:], 1.0)

                # v = (m - 2) * m; accum -> Sum_v   (v itself is dead)
                # Split v columns with ACT (which has slack) via
                # w = (m - 1)^2 = v + 1  =>  Sum_v = Sum_w - count.
                # Tail blocks split half/half: DVE is the drain bottleneck at
                # the end of the kernel while ACT idles.
                if b >= nblk - 2:
                    wcols = 1280
                elif b == nblk - 3:
                    wcols = D // 2
                else:
                    wcols = WCOL_STEADY
                split = D - wcols
                v = u_pool.tile([128, split], bf16, tag="v")
                nc.vector.scalar_tensor_tensor(
                    v[:], m[:, 0:split], 2.0, m[:, 0:split],
                    op0=Alu.subtract, op1=Alu.mult,
                    accum_out=stats_v[:, b:b + 1])
                if wcols:
                    w = u_pool.tile([128, wcols], bf16, tag="w")
                    nc.scalar.activation(
                        w[:], m[:, split:D], Act.Square, bias=negone[:],
                        accum_out=stats_w[:, b:b + 1])

            # partial = Sum_a + 0.5 * (Sum_v + Sum_w - wtot), per partition
            red_a = const_pool.tile([128, 1], f32, tag="red_a")
            nc.vector.tensor_reduce(red_a[:], stats_a[:],
                                    axis=mybir.AxisListType.X, op=Alu.add)
            red_v = const_pool.tile([128, 1], f32, tag="red_v")
            nc.vector.tensor_reduce(red_v[:], stats_v[:],
                                    axis=mybir.AxisListType.X, op=Alu.add)
            red_w = const_pool.tile([128, 1], f32, tag="red_w")
            nc.vector.tensor_reduce(red_w[:], stats_w[:],
                                    axis=mybir.AxisListType.X, op=Alu.add)
            wtot = WCOL_STEADY * (nblk - 3) + 2 * 1280 + D // 2
            red_vw = const_pool.tile([128, 1], f32, tag="red_vw")
            nc.vector.scalar_tensor_tensor(red_vw[:], red_w[:], -float(wtot),
                                           red_v[:], op0=Alu.add, op1=Alu.add)
            red = const_pool.tile([128, 1], f32, tag="red")
            nc.vector.scalar_tensor_tensor(red[:], red_vw[:], 0.5,
                                           red_a[:], op0=Alu.mult,
                                           op1=Alu.add)
            fin = psum_d_pool.tile([1, 1], f32, tag="d")
            nc.tensor.matmul(fin[:], red[:], ones1[:], start=True, stop=True)
            outsb = const_pool.tile([1, 1], f32, tag="outsb")
            nc.scalar.copy(outsb[:], fin[:])
            nc.sync.dma_start(out_d[:], outsb[:])

    nc.compile()
    return nc


_PROGRAM_CACHE = {}


def _get_program(nblk, D):
    key = (nblk, D)
    if key not in _PROGRAM_CACHE:
        _PROGRAM_CACHE[key] = _build_program(nblk, D)
    return _PROGRAM_CACHE[key]


def make_in_maps(feats, labels, cam_ids):
    """Host shard + program build; returns (nc, in_maps, N, D)."""
    feats_s, m_mat, nblk, N, D = _preprocess(feats, labels, cam_ids)
    nc = _get_program(nblk, D)
    in_maps = [
        {"feats_s": feats_s[c]}
        for c in range(N_CORES)
    ]
    return nc, in_maps, N, D


def kernel(feats, labels, cam_ids):
    from concourse.bass_utils import run_bass_kernel_spmd

    nc, in_maps, N, D = make_in_maps(feats, labels, cam_ids)
    res = run_bass_kernel_spmd(nc, in_maps, core_ids=list(range(N_CORES)))
    total = np.sum(
        np.array([res.results[c]["partial"][0, 0] for c in range(N_CORES)],
                 dtype=np.float64))
    return np.float32(total / (float(N) * float(D)))
